# revision 18
# baseline (speedup 1.0000x reference)
"""COPNLL loss kernel for Trainium2 (8 NeuronCores) — v3 (local-Gram AllReduce).

Math: the reference builds V = (sig2e*I + sig2bs0*Z0 Z0^T + sig2bs1*Z1 Z1^T)/sig2
with Z0 (4096x1000), Z1 (4096x500) one-hot, then needs logdet(V) and m^T V^-1 m.
Both reduce via Woodbury to the 1500x1500 capacitance matrix whose (0,0) block
is diagonal, leaving one dense 500x500 Schur complement
    S = (sig2e/s1*I + diag(c1)) - C^T diag(1/A) C,   A = sig2e/s0 + c0
with C = Z0^T Z1 (co-occurrence counts), c0/c1 level counts, a = Z0^T m, b = Z1^T m:
    logdet(sig2*V) = (N-q)log sig2e + q0 log s0 + q1 log s1 + sum(log A) + logdet S
    m^T V^-1 m     = (sig2/sig2e) * (m^T m - a^T A^-1 a - t^T S^-1 t),
                     t = b - C^T (a/A)

Device plan (SPMD, 8 cores, FULL inputs replicated to every core):
  phase A: core p owns the 126-level window [126p, 126p+126) of the level-0
    axis. Per 128-row chunk (32 chunks = all N rows) ONE matmul with the
    stationary matrix [onehot0_window | 1 | m] (128 cols) against
    [onehot1 | 1 | m] (502 cols) accumulates, over all rows:
      rows 0..125: C slice | counts0 slice | a slice
      rows 126/127: the full G1 = [1|m]^T[Z1|1|m]  (identical on all cores)
  local Gram (pre-collective): each core scales ITS C-window rows by
    1/sqrt(A) and computes its upper-triangle Gram contribution to
    W^T W (the 500x500 downdate) on the PE, plus per-partition logA and
    quad_a lanes. The t-vector partial rides as Gram column 500 (xw).
  comm: ONE f32 AllReduce(add) of the ~640KB Gram partials. After it, S is
    ready up to a negate + diagonal add — no per-tile reassembly.
  phase C (redundant on all cores): block LDL with 128-blocks stored
    upper-triangular; block inverses via 2-step Newton-Schulz with the
    X0 = alpha*I shortcut; block logdets via a degree-3 Chebyshev trace
    of log accumulated on the PE.
"""

import math
import sys
import types

import numpy as np

import concourse.bass as bass
import concourse.bacc as bacc
import concourse.mybir as mybir
from concourse.bass import ds, ts
from concourse.bass_utils import run_bass_kernel_spmd
from concourse.masks import make_identity
from concourse.tile import TileContext


def _ensure_axon_hooks():
    """bass_utils imports antenv.axon_hooks when tracing; this image's antenv
    lacks it. Provide a shim (with the real ctypes NTFF hook when available)
    so trace=True/BASS_TRACE never crashes the kernel."""
    try:
        import antenv.axon_hooks  # noqa: F401
        return
    except ImportError:
        pass
    try:
        import trn_agent_boot.trn_boot as tb
        hook = tb._ntff_profile_via_ctypes("/opt/axon/libaxon_pjrt.so")
    except Exception:
        hook = None
    mod = types.ModuleType("antenv.axon_hooks")
    mod._hook = hook
    mod.get_axon_ntff_profile_hook = lambda: mod._hook

    def _set(h):
        mod._hook = h

    mod.set_axon_ntff_profile_hook = _set
    sys.modules["antenv.axon_hooks"] = mod
    try:
        import antenv
        antenv.axon_hooks = mod
    except ImportError:
        pass
    try:
        import concourse.bass_utils as bu
        _orig_upload = bu.upload_artifacts

        def _safe_upload(tmpdir):
            try:
                return _orig_upload(tmpdir)
            except Exception:
                return f"local:{tmpdir}"

        bu.upload_artifacts = _safe_upload
    except Exception:
        pass


_ensure_axon_hooks()

N = 4096
NCORES = 8
NCH = N // 128             # 32 row chunks, every core sees all of them
WIN = 126                  # level-0 window width per core (8*126=1008 >= 1000)
Q0 = 1000
Q1 = 500
FR = Q1 + 2                # rhs width: [Z1 | 1 | m]
SP = 512                   # padded S size
NBLK = SP // 128           # 4
W3 = Q1 - 3 * 128          # 116: valid rows of the last S block
WS = [SP - 128 * k for k in range(NBLK)]       # stored row widths 512/384/256/128
OFFS = [0, 512, 896, 1152]                     # reduce-payload offsets
RW = OFFS[3] + WS[3] + 2                       # 1282 f32 per partition
LO, HI = 1.4, 18.0         # eigenvalue bounds for NS init + Chebyshev interval
NS_ITERS = 2
CHEB_DEG = 3
NCOEF = CHEB_DEG + 1
CLIP = 4.2648907939226017  # sqrt(2)*erfinv(1-2e-5)

F32 = mybir.dt.float32
BF16 = mybir.dt.bfloat16
I8 = mybir.dt.int8
I32 = mybir.dt.int32
U32 = mybir.dt.uint32
AX = mybir.AxisListType
OP = mybir.AluOpType
ACT = mybir.ActivationFunctionType


def cheb_coeffs(lo=LO, hi=HI, deg=CHEB_DEG):
    K = 4000
    th = (np.arange(K) + 0.5) * np.pi / K
    xk = np.cos(th)
    fk = np.log((hi - lo) / 2.0 * xk + (hi + lo) / 2.0)
    cs = np.array([2.0 / K * np.sum(fk * np.cos(j * th)) for j in range(deg + 1)])
    cs[0] *= 0.5
    return cs.astype(np.float32)


def _diag_fill(nc, tile_ap, value):
    nc.gpsimd.memset(tile_ap, 0.0)
    nc.gpsimd.affine_select(out=tile_ap, in_=tile_ap, compare_op=OP.not_equal,
                            fill=value, base=0, pattern=[[-1, 128]],
                            channel_multiplier=1)


def build_module(n_cores=NCORES):
    nc = bacc.Bacc(num_devices=n_cores)
    pk_d = nc.declare_dram_parameter("packed", [128, 4 * NCH], F32,
                                     isOutput=False)
    cst_d = nc.declare_dram_parameter("consts", [16], F32, isOutput=False)
    chb_d = nc.declare_dram_parameter("chebc", [2 * NCOEF], F32, isOutput=False)
    out_d = nc.declare_dram_parameter("out", [1, 1], F32, isOutput=True)

    red_in = nc.dram_tensor("red_in", [128 * RW], F32)
    red_out = nc.dram_tensor("red_out", [128 * RW], F32, addr_space="Shared")

    with TileContext(nc) as tc, \
         tc.tile_pool(name="consts", bufs=1) as consts, \
         tc.tile_pool(name="work", bufs=1) as work:

        # iotas first on gpsimd (before the affine_select diag fills) so the
        # one-hot compares aren't gated on a gpsimd library reload.
        iota1i = work.tile([128, Q1], I32, tag="iota1i")
        nc.gpsimd.iota(iota1i, pattern=[[1, Q1]], base=0, channel_multiplier=0)
        iotaWi = work.tile([128, WIN], I32, tag="iotaWi")
        nc.gpsimd.iota(iotaWi, pattern=[[1, WIN]], base=0, channel_multiplier=0)
        iotaPi = work.tile([128, 1], I32, tag="iotaPi")
        nc.gpsimd.iota(iotaPi, pattern=[[0, 1]], base=0, channel_multiplier=1)

        # ---- constants ----
        ident = consts.tile([128, 128], F32, tag="ident")
        make_identity(nc, ident)
        i2 = consts.tile([128, 128], F32, tag="i2")              # 2*I
        _diag_fill(nc, i2, 2.0)
        shiftI = consts.tile([128, 128], F32, tag="shiftI")      # Chebyshev shift
        _diag_fill(nc, shiftI, (HI + LO) / (HI - LO))
        ones512 = consts.tile([128, SP], F32, tag="ones512")
        nc.vector.memset(ones512, 1.0)
        identB16 = consts.tile([128, 128], BF16, tag="identB16")
        nc.vector.tensor_copy(identB16, ident)
        cst_row = consts.tile([1, 16], F32, tag="cst_row")
        nc.sync.dma_start(cst_row, cst_d[:].rearrange("(p x) -> p x", p=1))
        cst_row2 = consts.tile([1, 16], F32, tag="cst_row2")
        nc.vector.tensor_copy(cst_row2, cst_row)
        chb = consts.tile([1, 2 * NCOEF], F32, tag="chb")
        nc.sync.dma_start(chb, chb_d[:].rearrange("(p x) -> p x", p=1))
        chb2 = consts.tile([1, 2 * NCOEF], F32, tag="chb2")
        nc.vector.tensor_copy(chb2, chb)
        cst = consts.tile([128, 16], F32, tag="cst")
        chbB = consts.tile([128, 2 * NCOEF], F32, tag="chbB")
        with tc.tile_pool(name="setup_ps", bufs=2,
                          space=bass.MemorySpace.PSUM) as gps0:
            ps_b = gps0.tile([128, 16], F32, tag="gps0")
            nc.tensor.matmul(ps_b, ones512[0:1, 0:128], cst_row2,
                             start=True, stop=True)
            nc.vector.tensor_copy(cst, ps_b)
            ps_c = gps0.tile([128, 2 * NCOEF], F32, tag="gps0")
            nc.tensor.matmul(ps_c, ones512[0:1, 0:128], chb2,
                             start=True, stop=True)
            nc.vector.tensor_copy(chbB, ps_c)

        # f32 iotas (bf16 can't represent odd ints >= 257; f32-in/bf16-out is
        # also the DVE's measured-fast compare path)
        iota1 = work.tile([128, Q1], F32, tag="iota1")
        nc.vector.tensor_copy(iota1, iota1i)
        iotaW = work.tile([128, WIN], F32, tag="iotaW")
        nc.vector.tensor_copy(iotaW, iotaWi)
        # shift by this core's window base (per-core const c[9])
        nc.vector.tensor_scalar(out=iotaW, in0=iotaW, scalar1=cst[:, 9:10],
                                scalar2=None, op0=OP.add)
        # local row-validity masks: valid = 2 <= p <= thr (c[10]); level =
        # 126*core + p - 2. pm3 marks the S pad rows of block 3 (p > 115).
        iotaP = work.tile([128, 1], F32, tag="iotaP")
        nc.vector.tensor_copy(iotaP, iotaPi)
        vml = work.tile([128, 1], F32, tag="vml")
        nc.vector.tensor_scalar(out=vml, in0=iotaP, scalar1=cst[:, 10:11],
                                scalar2=None, op0=OP.is_le)
        lowm = work.tile([128, 1], F32, tag="lowm")
        nc.vector.tensor_scalar(out=lowm, in0=iotaP, scalar1=1.5,
                                scalar2=None, op0=OP.is_gt)
        nc.vector.tensor_mul(vml, vml, lowm)
        padl = work.tile([128, 1], U32, tag="padl")
        nc.vector.tensor_scalar(out=padl, in0=vml, scalar1=-1.0,
                                scalar2=1.0, op0=OP.mult, op1=OP.add)
        pm3 = work.tile([128, 1], U32, tag="pm3")
        nc.vector.tensor_scalar(out=pm3, in0=iotaP,
                                scalar1=float(W3) - 0.5,
                                scalar2=None, op0=OP.is_gt)
        # [128,128] wide version of pm3 for the block-3 pad-row overwrite
        iotaPw = work.tile([128, 128], I32, tag="iotaPw")
        nc.gpsimd.iota(iotaPw, pattern=[[0, 128]], base=0, channel_multiplier=1)
        pm3w = work.tile([128, 128], U32, tag="pm3w")
        nc.vector.tensor_scalar(out=pm3w, in0=iotaPw,
                                scalar1=float(W3) - 0.5,
                                scalar2=None, op0=OP.is_gt)

        # ---- inputs -> m, sum r^2, sum m^2 ----
        packed = work.tile([128, 4 * NCH], F32, tag="packed")
        nc.sync.dma_start(packed, pk_d[:])
        yt = packed[:, 0:NCH]
        yp = packed[:, NCH:2 * NCH]
        idx0 = work.tile([128, NCH], F32, tag="idx0")
        nc.vector.tensor_copy(idx0, packed[:, 2 * NCH:3 * NCH].bitcast(I32))
        idx1 = work.tile([128, NCH], F32, tag="idx1")
        nc.vector.tensor_copy(idx1, packed[:, 3 * NCH:4 * NCH].bitcast(I32))
        resid = work.tile([128, NCH], F32, tag="resid")
        nc.vector.tensor_sub(resid, yt, yp)
        mvec = work.tile([128, NCH], F32, tag="mvec")
        nc.vector.tensor_scalar(out=mvec, in0=resid, scalar1=cst[:, 0:1],
                                scalar2=cst[:, 1:2], op0=OP.mult, op1=OP.min)
        nc.vector.tensor_scalar(out=mvec, in0=mvec, scalar1=cst[:, 8:9],
                                scalar2=None, op0=OP.max)
        scr_n = work.tile([128, NCH], F32, tag="scr_n")
        scal2 = work.tile([128, 2], F32, tag="scal2")
        nc.vector.tensor_mul(scr_n, resid, resid)
        nc.vector.tensor_reduce(scal2[:, 0:1], scr_n, AX.X, OP.add)
        nc.vector.tensor_mul(scr_n, mvec, mvec)
        nc.vector.tensor_reduce(scal2[:, 1:2], scr_n, AX.X, OP.add)
        scal2r = work.tile([1, 2], F32, tag="scal2r")
        with tc.tile_pool(name="sc_ps", bufs=1,
                          space=bass.MemorySpace.PSUM) as gpsc:
            ps_s = gpsc.tile([128, 2], F32, tag="gpsc")
            nc.tensor.matmul(ps_s[0:1, 0:2], ones512[:, 0:1], scal2,
                             start=True, stop=True)
            nc.vector.tensor_copy(scal2r, ps_s[0:1, 0:2])

        # ---- phase A + local Gram ----
        # 3 persistent oh/rh buffers rotate (pipeline depth 3); their constant
        # ones-columns are written ONCE. Both one-hot compares stay on the DVE
        # (f32-in/bf16-out is its fast path; gpsimd and fp16 variants measured
        # 4-8x slower); m copies go to the scalar engine.
        G1 = work.tile([2, FR], F32, tag="G1")
        oh3 = [work.tile([128, 128], BF16, tag=f"oh3_{j}", name=f"oh3_{j}")
               for j in range(3)]
        rh3 = [work.tile([128, FR], BF16, tag=f"rh3_{j}", name=f"rh3_{j}")
               for j in range(3)]
        for j in range(3):
            nc.vector.memset(oh3[j][:, 0:1], 1.0)
            nc.vector.memset(rh3[j][:, Q1:Q1 + 1], 1.0)
        Avec = work.tile([128, 1], F32, tag="Avec")
        Winv = work.tile([128, 1], F32, tag="Winv")
        wsq = work.tile([128, 1], F32, tag="wsq")
        xw = work.tile([128, 1], F32, tag="xw")
        ex2 = work.tile([128, 2], F32, tag="ex2")
        rhw = work.tile([128, SP], BF16, tag="rhw")
        nc.vector.memset(rhw[:, Q1 + 1:SP], 0.0)
        with tc.tile_pool(name="phA_ps", bufs=1,
                          space=bass.MemorySpace.PSUM) as pps:
            psC = pps.tile([128, FR], F32, tag="psC")
            for c in range(NCH):
                # cols 0/1 = [1|m] (G1 lands at psC rows 0/1: PSUM partition
                # reads must start at 0); cols 2..127 = level-window one-hot
                oh = oh3[c % 3]
                rh = rh3[c % 3]
                nc.scalar.copy(oh[:, 1:2], mvec[:, c:c + 1])
                nc.vector.tensor_scalar(out=oh[:, 2:128], in0=iotaW,
                                        scalar1=idx0[:, c:c + 1],
                                        scalar2=None, op0=OP.is_equal)
                nc.vector.tensor_scalar(out=rh[:, 0:Q1], in0=iota1,
                                        scalar1=idx1[:, c:c + 1],
                                        scalar2=None, op0=OP.is_equal)
                nc.scalar.copy(rh[:, Q1 + 1:FR], mvec[:, c:c + 1])
                nc.tensor.matmul(psC, oh, rh, start=(c == 0),
                                 stop=(c == NCH - 1))
            # G1 readout + the local A -> 1/sqrt(A) chain (rows 0/1 and the
            # level>=1000 pad rows get ws=0 via vml, so they vanish from the
            # Gram; invalid A is forced to 1 so its log is 0)
            nc.vector.tensor_copy(G1, psC[0:2, :])
            nc.vector.tensor_scalar(out=Avec, in0=psC[:, Q1:Q1 + 1],
                                    scalar1=cst[:, 2:3],
                                    scalar2=None, op0=OP.add)
            nc.vector.copy_predicated(Avec, padl, ones512[:, 0:1])
            nc.vector.reciprocal(Winv, Avec)
            nc.vector.tensor_mul(Winv, Winv, vml)
            nc.scalar.sqrt(wsq, Winv)
            nc.vector.tensor_tensor(out=xw, in0=psC[:, Q1 + 1:FR], in1=wsq,
                                    op=OP.mult)
            nc.scalar.activation(ex2[:, 1:2], Avec, ACT.Ln)
            nc.vector.tensor_mul(ex2[:, 0:1], xw, xw)
            nc.vector.tensor_scalar_mul(rhw[:, 0:Q1], psC[:, 0:Q1], wsq)
            nc.vector.tensor_copy(rhw[:, Q1:Q1 + 1], xw)

        # local upper-triangle Gram contribution (+ qa/logA lanes on block 3),
        # staged through SBUF (DMA can't read PSUM) and sent in one DMA
        stg2 = work.tile([128, RW], F32, tag="stg2")
        with tc.tile_pool(name="gram_ps", bufs=1,
                          space=bass.MemorySpace.PSUM) as sps2:
            psS2 = [sps2.tile([128, WS[k]], F32,
                              tag=f"psS2_{k}", name=f"psS2_{k}")
                    for k in range(NBLK)]
            for k in range(NBLK):
                nc.tensor.matmul(psS2[k], rhw[:, ds(128 * k, 128)],
                                 rhw[:, 128 * k:SP], start=True, stop=True)
                nc.vector.tensor_copy(stg2[:, OFFS[k]:OFFS[k] + WS[k]],
                                      psS2[k])
            nc.vector.tensor_copy(
                stg2[:, OFFS[NBLK - 1] + 128:OFFS[NBLK - 1] + 130], ex2)
        rv_in = red_in[:].rearrange("(p f) -> p f", p=128)
        nc.sync.dma_start(rv_in[:, 0:RW], stg2)

        if n_cores > 1:
            nc.gpsimd.collective_compute(
                "AllReduce", OP.add,
                replica_groups=[list(range(n_cores))],
                ins=[red_in[:]], outs=[red_out[:]],
            )
        else:
            nc.sync.dma_start(red_out[:], red_in[:])

        # ---- phase C constants built while the collective is in flight ----
        alpha = 2.0 / (LO + HI)
        cheb_sc = 2.0 / (HI - LO)
        cbt = [work.tile([128, 2], F32, tag=f"cb{i}", name=f"cb{i}")
               for i in range(NBLK)]
        dSl = [work.tile([128, 1], F32, tag=f"dS{i}", name=f"dS{i}")
               for i in range(NBLK)]
        dgN = [work.tile([128, 128], BF16, tag=f"dg{i}", name=f"dg{i}")
               for i in range(NBLK)]
        with tc.tile_pool(name="g1_ps", bufs=2,
                          space=bass.MemorySpace.PSUM) as gps1:
            for i in range(NBLK):
                wi = 128 if i < NBLK - 1 else W3
                psT = gps1.tile([128, 2], F32, tag="pst")
                nc.tensor.transpose(psT[:wi, :], G1[0:2, ds(i * 128, wi)],
                                    ident[0:2, 0:2])
                nc.vector.memset(cbt[i], 0.0)
                nc.vector.tensor_copy(cbt[i][:wi, :], psT[:wi, :])
                nc.vector.tensor_scalar(out=dSl[i], in0=cbt[i][:, 0:1],
                                        scalar1=cst[:, 3:4],
                                        scalar2=None, op0=OP.add)
                if i == NBLK - 1:
                    nc.vector.copy_predicated(dSl[i], pm3, ones512[:, 0:1])
                nc.vector.tensor_scalar_mul(dgN[i], ident, dSl[i])
        cI = []
        for j in range(NCOEF):
            cj = work.tile([128, 128], BF16, tag=f"cI{j}", name=f"cI{j}")
            nc.vector.tensor_scalar_mul(cj, ident, chbB[:, j:j + 1])
            cI.append(cj)

        # ---- S from the reduced Gram: Srow[k] = dg_k*I - gram_k ----
        # (upper-triangular storage: row-block k holds global cols
        # [128k, 512); its diagonal block is local cols 0..128; the t-vector
        # partial sits at local col 500-128k and is zeroed out of S)
        Srow = [work.tile([128, WS[i]], BF16, tag=f"Srow{i}", name=f"Srow{i}")
                for i in range(NBLK)]
        zvec = [work.tile([128, 1], F32, tag=f"z{i}", name=f"z{i}")
                for i in range(NBLK)]
        Wk = [work.tile([128, SP - (k + 1) * 128], BF16, tag=f"Wk{k}",
                        name=f"Wk{k}") for k in range(NBLK - 1)]
        trc = work.tile([128, NBLK], F32, tag="trc")
        qtt = work.tile([128, NBLK], F32, tag="qtt")

        gramk = [work.tile([128, WS[k] + (2 if k == NBLK - 1 else 0)], F32,
                           tag=f"gram{k}", name=f"gram{k}")
                 for k in range(NBLK)]
        rv_out = red_out[:].rearrange("(p f) -> p f", p=128)
        for k in range(NBLK):
            w = WS[k] + (2 if k == NBLK - 1 else 0)
            deng = nc.sync if k % 2 == 0 else nc.gpsimd
            deng.dma_start(gramk[k], rv_out[:, OFFS[k]:OFFS[k] + w])
        for k in range(NBLK):
            tcol = Q1 - 128 * k
            wi = 128 if k < NBLK - 1 else W3
            nc.vector.tensor_scalar_mul(Srow[k], gramk[k][:, 0:WS[k]], -1.0)
            nc.vector.tensor_add(Srow[k][:, 0:128], Srow[k][:, 0:128], dgN[k])
            if k < NBLK - 1:
                nc.vector.memset(Srow[k][:, tcol:tcol + 1], 0.0)
            else:
                nc.vector.memset(Srow[k][0:W3, tcol:tcol + 1], 0.0)
                # pad rows (>=116) of the last block become identity rows
                # (this also scrubs the xw-Gram artifacts from row 116)
                nc.vector.copy_predicated(Srow[k][:, 0:128], pm3w, identB16)
            nc.vector.memset(zvec[k], 0.0)
            nc.vector.tensor_sub(zvec[k][:wi, :], cbt[k][:wi, 1:2],
                                 gramk[k][:wi, tcol:tcol + 1])

        with (
            tc.tile_pool(name="ldl", bufs=4) as ldl,
            tc.tile_pool(name="ldl_ps", bufs=4,
                         space=bass.MemorySpace.PSUM) as lps,
            tc.tile_pool(name="rps_pool", bufs=1,
                         space=bass.MemorySpace.PSUM) as rpsp,
        ):
            # ---- block LDL (Hotelling/NS inverses) with the Chebyshev
            # trace recurrences emission-interleaved into the NS bubbles.
            cheb = {}          # k -> [b2, tprev, tcur, next_j]
            pending = []
            Rps = []

            def cheb_start(k):
                bh = ldl.tile([128, 128], BF16, tag=f"bh{k}", name=f"bh{k}")
                Bk = Srow[k][:, 0:128]
                nc.vector.tensor_scalar_mul(bh, Bk, cheb_sc)
                nc.vector.tensor_sub(bh, bh, shiftI)
                b2 = ldl.tile([128, 128], BF16, tag=f"b2{k}", name=f"b2{k}")
                nc.vector.tensor_scalar_mul(b2, bh, 2.0)
                R = rpsp.tile([128, 128], F32, tag=f"Rps{k}", name=f"Rps{k}")
                Rps.append(R)
                nc.tensor.matmul(R, cI[1], bh, start=True, stop=False)
                cheb[k] = [b2, identB16, bh, 2]
                pending.append(k)

            def cheb_round():
                if not pending:
                    return
                k = pending.pop(0)
                b2, tprev, tcur, j = cheb[k]
                psc = lps.tile([128, 128], F32, tag="lps")
                nc.tensor.matmul(psc, b2, tcur, start=True, stop=True)
                tnext = ldl.tile([128, 128], BF16, tag=f"chT{k}",
                                 name=f"chT{k}_{j}", bufs=3)
                nc.vector.tensor_sub(tnext, psc, tprev)
                nc.tensor.matmul(Rps[k], cI[j], tnext, start=False,
                                 stop=(j == CHEB_DEG))
                cheb[k] = [b2, tcur, tnext, j + 1]
                if j + 1 <= CHEB_DEG:
                    pending.append(k)
                else:
                    Rsb = ldl.tile([128, 128], F32, tag="Rsb")
                    nc.vector.tensor_mul(Rsb, Rps[k], ident)
                    nc.vector.tensor_reduce(trc[:, k:k + 1], Rsb, AX.X, OP.add)

            # 2-iteration Newton-Schulz with the X0 = alpha*I shortcut:
            #   X1 = alpha*Z0 (no matmul), X2 = X1 Z1 = alpha * Z0 Z1.
            # All iterates are polynomials of B -> symmetric -> the lhsT
            # transpose in matmul is harmless.
            Xfin = []
            for k in range(NBLK):
                # D_k is final here (panel k-1 updates already applied)
                cheb_start(k)
                trail = WS[k] - 128
                Y = ldl.tile([128, 128], BF16, tag="nsY")
                nc.vector.tensor_scalar_mul(Y, Srow[k][:, 0:128], alpha)
                Z0 = ldl.tile([128, 128], BF16, tag="nsZ")
                nc.vector.tensor_sub(Z0, i2, Y)
                psY = lps.tile([128, 128], F32, tag="lps")
                nc.tensor.matmul(psY, Y, Z0, start=True, stop=True)
                cheb_round()
                Z1 = ldl.tile([128, 128], BF16, tag="nsZ")
                nc.vector.tensor_sub(Z1, i2, psY)
                psX = lps.tile([128, 128], F32, tag="lps")
                nc.tensor.matmul(psX, Z0, Z1, start=True, stop=True)
                cheb_round()
                X = ldl.tile([128, 128], BF16, tag=f"nsXf{k}", name=f"nsXf{k}")
                nc.vector.tensor_scalar(out=X, in0=psX, scalar1=float(alpha),
                                        scalar2=None, op0=OP.mult)
                Xfin.append(X)
                if trail:
                    psW = lps.tile([128, 384], F32, tag="lps")
                    nc.tensor.matmul(psW[:, :trail], X, Srow[k][:, 128:WS[k]],
                                     start=True, stop=True)
                    nc.vector.tensor_copy(Wk[k], psW[:, :trail])
                    for i in range(k + 1, NBLK):
                        # update row-block i on its whole stored width
                        woff = 128 * (i - k - 1)
                        psu = lps.tile([128, 384], F32, tag="lps")
                        nc.tensor.matmul(psu[:, :WS[i]],
                                         Srow[k][:, ds(128 * (i - k), 128)],
                                         Wk[k][:, woff:woff + WS[i]],
                                         start=True, stop=True)
                        nc.vector.tensor_sub(Srow[i], Srow[i],
                                             psu[:, :WS[i]])
                    cheb_round()
                    cheb_round()
            while pending:
                cheb_round()

            # forward substitution: z_i -= (Wk[k] block i)^T z_k
            # (fp16 Wk x bf16 z-cast; zvec itself stays f32)
            zh = [None] * NBLK
            for k in range(NBLK - 1):
                zh[k] = ldl.tile([128, 1], BF16, tag=f"zh{k}", name=f"zh{k}")
                nc.vector.tensor_copy(zh[k], zvec[k])
                for i in range(k + 1, NBLK):
                    psz = lps.tile([128, 1], F32, tag="lps")
                    off = (i - k - 1) * 128
                    nc.tensor.matmul(psz, Wk[k][:, ds(off, 128)], zh[k],
                                     start=True, stop=True)
                    nc.vector.tensor_sub(zvec[i], zvec[i], psz)
            zh[NBLK - 1] = ldl.tile([128, 1], BF16, tag=f"zh{NBLK - 1}",
                                    name=f"zh{NBLK - 1}")
            nc.vector.tensor_copy(zh[NBLK - 1], zvec[NBLK - 1])
            # quad_t = sum_k z_k^T Binv_k z_k
            for k in range(NBLK):
                psq = lps.tile([128, 1], F32, tag="lps")
                nc.tensor.matmul(psq, Xfin[k], zh[k], start=True, stop=True)
                uk = ldl.tile([128, 1], F32, tag="uk")
                nc.vector.tensor_copy(uk, psq)
                nc.vector.tensor_mul(qtt[:, k:k + 1], zvec[k], uk)

        # ---- final scalar assembly ----
        qtr = work.tile([128, 1], F32, tag="qtr")
        nc.vector.tensor_reduce(qtr, qtt, AX.X, OP.add)
        smalls_c = work.tile([128, 3 + NBLK], F32, tag="smalls_c")
        nc.vector.tensor_copy(smalls_c[:, 0:1], gramk[NBLK - 1][:, 129:130])
        nc.vector.tensor_copy(smalls_c[:, 1:2], gramk[NBLK - 1][:, 128:129])
        nc.vector.tensor_copy(smalls_c[:, 2:3], qtr)
        nc.vector.tensor_copy(smalls_c[:, 3:3 + NBLK], trc)
        smalls = work.tile([1, 3 + NBLK], F32, tag="smalls")
        ldS = work.tile([1, 1], F32, tag="ldS")
        with tc.tile_pool(name="fin_ps", bufs=1,
                          space=bass.MemorySpace.PSUM) as gps2:
            ps_sm = gps2.tile([128, 3 + NBLK], F32, tag="gps2")
            nc.tensor.matmul(ps_sm[0:1, :], ones512[:, 0:1], smalls_c,
                             start=True, stop=True)
            nc.vector.tensor_copy(smalls, ps_sm[0:1, :])
        nc.vector.tensor_reduce(ldS, smalls[:, 3:3 + NBLK], AX.X, OP.add)

        fin = work.tile([1, 8], F32, tag="fin")
        mtm = scal2r[:, 1:2]
        r2g = scal2r[:, 0:1]
        # quadK = quad_a + quad_t
        nc.vector.tensor_add(fin[:, 0:1], smalls[:, 1:2], smalls[:, 2:3])
        # mVinvm = (sig2/sig2e) * (mtm - quadK)
        nc.vector.tensor_sub(fin[:, 1:2], mtm, fin[:, 0:1])
        nc.vector.tensor_scalar_mul(fin[:, 1:2], fin[:, 1:2], cst[0:1, 6:7])
        # logdetV = const1 + sum log A + logdet S
        nc.vector.tensor_add(fin[:, 2:3], smalls[:, 0:1], ldS)
        nc.vector.tensor_scalar(out=fin[:, 2:3], in0=fin[:, 2:3],
                                scalar1=cst[0:1, 4:5], scalar2=None, op0=OP.add)
        # sum_log_pdf = const2 - sum_r2/(2 sig2)
        nc.vector.tensor_scalar(out=fin[:, 3:4], in0=r2g, scalar1=cst[0:1, 7:8],
                                scalar2=cst[0:1, 5:6], op0=OP.mult, op1=OP.add)
        # total = 0.5*(logdetV + mVinvm - mtm + sum_log_pdf)
        nc.vector.tensor_add(fin[:, 4:5], fin[:, 2:3], fin[:, 1:2])
        nc.vector.tensor_sub(fin[:, 4:5], fin[:, 4:5], mtm)
        nc.vector.tensor_add(fin[:, 4:5], fin[:, 4:5], fin[:, 3:4])
        nc.vector.tensor_scalar_mul(fin[:, 4:5], fin[:, 4:5], 0.5)

        nc.sync.dma_start(out_d[:], fin[:, 4:5])

    nc.finalize()
    return nc


def host_consts(sig2e, sig2bs, core_id):
    s0, s1 = float(sig2bs[0]), float(sig2bs[1])
    sig2e = float(sig2e)
    sig2 = sig2e + s0 + s1
    c = np.zeros(16, np.float32)
    c[0] = 1.0 / math.sqrt(sig2)
    c[1] = CLIP
    c[2] = sig2e / s0
    c[3] = sig2e / s1
    # the j=0 Chebyshev trace term (c_0 * 128 per block) folds in here
    c[4] = ((N - Q0 - Q1) * math.log(sig2e) + Q0 * math.log(s0)
            + Q1 * math.log(s1) - N * math.log(sig2)
            + NBLK * 128 * float(cheb_coeffs()[0]))
    c[5] = -0.5 * N * math.log(2.0 * math.pi * sig2)
    c[6] = sig2 / sig2e
    c[7] = -1.0 / (2.0 * sig2)
    c[8] = -CLIP
    c[9] = float(WIN * core_id)
    c[10] = float(Q0 - core_id * WIN) + 1.5 if core_id == NCORES - 1 else 127.5
    return c


_CACHE = {}


def _get_module(n_cores=NCORES):
    if n_cores not in _CACHE:
        _CACHE[n_cores] = build_module(n_cores)
    return _CACHE[n_cores]


def make_in_maps(inputs, n_cores=NCORES):
    y_true = np.ascontiguousarray(np.asarray(inputs["y_true"], np.float32).reshape(N, 1))
    y_pred = np.ascontiguousarray(np.asarray(inputs["y_pred"], np.float32).reshape(N, 1))
    zi0 = np.ascontiguousarray(np.asarray(inputs["Z_idx0"]).astype(np.int32).reshape(N))
    zi1 = np.ascontiguousarray(np.asarray(inputs["Z_idx1"]).astype(np.int32).reshape(N))
    cs = cheb_coeffs().astype(np.float32)
    chebc = np.concatenate([cs, np.zeros(NCOEF, np.float32)])
    pk = np.concatenate([
        y_true.reshape(NCH, 128).T,
        y_pred.reshape(NCH, 128).T,
        zi0.reshape(NCH, 128).T.view(np.float32),
        zi1.reshape(NCH, 128).T.view(np.float32),
    ], axis=1)
    pk = np.ascontiguousarray(pk)
    maps = []
    for i in range(n_cores):
        c = host_consts(np.asarray(inputs["sig2e"]),
                        np.asarray(inputs["sig2bs"], np.float64), i)
        maps.append({"packed": pk, "consts": c, "chebc": chebc})
    return maps


def kernel(**inputs):
    nc = _get_module(NCORES)
    maps = make_in_maps(inputs, NCORES)
    res = run_bass_kernel_spmd(nc, maps, list(range(NCORES)))
    out = np.asarray(res.results[0]["out"], np.float32).reshape(1, 1)
    return out
